# revision 3
# baseline (speedup 1.0000x reference)
"""MinGRU layer on 8 Trainium2 NeuronCores (batch-parallel).

Math (per batch b, reference semantics, all fp32):
    g = sigmoid(x @ Wg.T + bg)
    a = sigmoid(x @ Wd.T + bd)
    v = x @ Wv.T + bv
    h_t = a_t * h_{t-1} + (1 - a_t) * v_t     (causal scan over S)
    out = h * g

Design:
  - B=8 batches -> one batch element per NeuronCore (embarrassingly parallel).
  - Host side: x[b] is transposed to X.T [D, S] and cast to bf16; weights are
    transposed to W.T [d, e] layout and cast to bf16 (matmul operands).
  - On-chip per core:
      * matmuls produce projections directly in [e(part), s(free)] layout:
        out[e, s] = sum_d W.T[d, e] * X.T[d, s]  via PE (stationary = W.T tile)
      * ACT evicts PSUM with fused bias+sigmoid (gate/decay) or bias+identity (v)
      * DVE computes u' = (a-1)*v, then the recurrence via the hardware
        TensorTensorScanArith op: state = a*state - u' = a*state + (1-a)*v
      * DVE computes h*g, DMA back to DRAM in [e, s]; host transposes to [s, e].
"""

import os

import numpy as np
import ml_dtypes

B, S, D = 8, 4096, 1024
P = 128
KT = D // P          # 8 contraction tiles
ET = D // P          # 8 output-channel tiles
SCH = 512            # psum free-dim chunk (one bank, fp32)
NS = S // SCH        # 8 s-chunks

_BF16 = ml_dtypes.bfloat16

_nc_cache = {}


def _build_nc():
    """Build + compile the single-core Bass program (shared by all 8 cores)."""
    if "nc" in _nc_cache:
        return _nc_cache["nc"]

    from contextlib import ExitStack

    import concourse.bacc as bacc
    import concourse.mybir as mybir
    from concourse import tile

    dt = mybir.dt
    AF = mybir.ActivationFunctionType
    OP = mybir.AluOpType

    nc = bacc.Bacc("TRN2", target_bir_lowering=False, debug=False, num_devices=8)

    xt = nc.dram_tensor("xt", [D, S], dt.bfloat16, kind="ExternalInput").ap()
    # wt[j] = W_j.T in [d, e] layout; j: 0=decay(Wd), 1=value(Wv), 2=gate(Wg)
    wt = nc.dram_tensor("wt", [3, D, D], dt.bfloat16, kind="ExternalInput").ap()
    # bias rows: 0=bd, 1=bv, 2=bg, 3=-bv  (bv is folded into the scan via the
    # substitution h = h' + bv: h' scans (a-1)*v with initial -bv, and the
    # output is (h' + bv) * g)
    bias = nc.dram_tensor("bias", [4, D], dt.float32, kind="ExternalInput").ap()
    out = nc.dram_tensor("out", [D, S], dt.float32, kind="ExternalOutput").ap()

    with tile.TileContext(nc) as tc, ExitStack() as ctx:
        xp = ctx.enter_context(tc.tile_pool(name="xp", bufs=1))
        wp = ctx.enter_context(tc.tile_pool(name="wp", bufs=1))
        bp = ctx.enter_context(tc.tile_pool(name="bp", bufs=1))
        work = ctx.enter_context(tc.tile_pool(name="work", bufs=1))
        outp = ctx.enter_context(tc.tile_pool(name="outp", bufs=1))
        psum = ctx.enter_context(tc.tile_pool(name="psum", bufs=1, space="PSUM"))

        # Biases: [p, j*ET + e] with bias[j, e*128 + p] at column j*ET+e
        btile = bp.tile([P, 4 * ET], dt.float32)
        nc.sync.dma_start(btile[:], bias.rearrange("j (e p) -> p (j e)", p=P))

        # Resident tiles, DMA'd in first-use order so the PE can start almost
        # immediately: projection order below is gate(2), decay(0), value(1);
        # within the first projection the k-loop consumes w2[k] + xt[k] pairs.
        xtiles = [None] * KT
        wtiles = [[None] * KT for _ in range(3)]

        def _load_w(j, k):
            t = wp.tile([P, D], dt.bfloat16, tag=f"w{j}_{k}", name=f"w{j}_{k}")
            nc.sync.dma_start(t[:], wt[j, k * P:(k + 1) * P, :])
            wtiles[j][k] = t

        for k in range(KT):
            _load_w(0, k)
            t = xp.tile([P, S], dt.bfloat16, tag=f"x{k}", name=f"x{k}")
            nc.sync.dma_start(t[:], xt[k * P:(k + 1) * P, :])
            xtiles[k] = t
        for k in range(KT):
            _load_w(1, k)
        for k in range(KT):
            _load_w(2, k)

        for et in range(ET):
            a = work.tile([P, S], dt.float32, tag="a")
            g = work.tile([P, S], dt.float32, tag="g")
            u = work.tile([P, S], dt.float32, tag="u")
            h = work.tile([P, S], dt.float32, tag="h")
            om = outp.tile([P, S], dt.float32, tag="om")

            # For e-tiles 0..ET-2 project in order decay(0), value(1), gate(2):
            # the next e-tile's first eviction (decay -> a) only WARs against
            # this tile's scan, which finishes early, so the PE never stalls.
            # The last e-tile instead runs gate first and a chunked scan
            # pipeline to minimize the kernel tail (no successor to stall).
            last = et == ET - 1
            for j in ((2, 0, 1) if last else (0, 1, 2)):
                ps = [
                    psum.tile([P, SCH], dt.float32, tag=f"ps{s}", name=f"ps{s}_{et}_{j}")
                    for s in range(NS)
                ]
                for k in range(KT):
                    lhsT = wtiles[j][k][:, et * P:(et + 1) * P]
                    for s in range(NS):
                        nc.tensor.matmul(
                            ps[s][:],
                            lhsT,
                            xtiles[k][:, s * SCH:(s + 1) * SCH],
                            start=(k == 0),
                            stop=(k == KT - 1),
                        )
                bcol = btile[:, j * ET + et: j * ET + et + 1]
                if j == 1:
                    # u' = (a-1) * v, consumed straight out of PSUM per bank —
                    # no ACT eviction for the value projection.
                    for s in range(NS):
                        sl = slice(s * SCH, (s + 1) * SCH)
                        nc.vector.scalar_tensor_tensor(
                            u[:, sl], a[:, sl], 1.0, ps[s][:],
                            op0=OP.subtract, op1=OP.mult,
                        )
                else:
                    dst = a if j == 0 else g
                    for s in range(NS):
                        sl = slice(s * SCH, (s + 1) * SCH)
                        nc.scalar.activation(dst[:, sl], ps[s][:], AF.Sigmoid, bias=bcol)

            nbv = btile[:, 3 * ET + et: 3 * ET + et + 1]   # -bv
            pbv = btile[:, 1 * ET + et: 1 * ET + et + 1]   # +bv
            if not last:
                # h'_t = a_t * h'_{t-1} - u'_t, h'_{-1} = -bv
                nc.vector.tensor_tensor_scan(
                    h[:], a[:], u[:], nbv, op0=OP.mult, op1=OP.subtract
                )
                # out = (h' + bv) * g
                nc.vector.scalar_tensor_tensor(
                    om[:], h[:], pbv, g[:], op0=OP.add, op1=OP.mult
                )
                nc.sync.dma_start(out[et * P:(et + 1) * P, :], om[:])
            else:
                # Chunked pipeline (4 x 1024) so eviction/scan/mul/store of
                # successive chunks overlap, shrinking the kernel tail.
                CW = 2 * SCH
                for c in range(NS // 2):
                    sl = slice(c * CW, (c + 1) * CW)
                    init = nbv if c == 0 else h[:, c * CW - 1: c * CW]
                    nc.vector.tensor_tensor_scan(
                        h[:, sl], a[:, sl], u[:, sl], init, op0=OP.mult, op1=OP.subtract
                    )
                    nc.vector.scalar_tensor_tensor(
                        om[:, sl], h[:, sl], pbv, g[:, sl], op0=OP.add, op1=OP.mult
                    )
                    nc.sync.dma_start(out[et * P:(et + 1) * P, sl], om[:, sl])

    nc.compile()
    _nc_cache["nc"] = nc
    return nc


def _start_trace():
    """Begin an NRT/NTFF profile capture on core 0 via the axon PJRT .so.

    Dev-only (MINGRU_TRACE=1); returns None on any failure so the normal
    execution path is never affected.
    """
    try:
        import ctypes
        import tempfile

        so = "/opt/axon/libaxon_pjrt.so"
        lib = ctypes.CDLL(so)
        if not hasattr(lib, "axon_start_nrt_profile"):
            return None
        lib.axon_start_nrt_profile.argtypes = [
            ctypes.POINTER(ctypes.c_int64),
            ctypes.c_size_t,
        ]
        lib.axon_start_nrt_profile.restype = ctypes.c_int64
        lib.axon_stop_nrt_profile.argtypes = [ctypes.c_char_p]
        lib.axon_stop_nrt_profile.restype = ctypes.c_int64

        import jax

        jax.devices()
        ids = (ctypes.c_int64 * 1)(0)
        rc = lib.axon_start_nrt_profile(ids, 1)
        if rc != 0:
            print(f"trace: axon_start_nrt_profile rc={rc}")
            return None
        outdir = tempfile.mkdtemp(prefix="mingru_ntff_")
        return (lib, outdir)
    except Exception as e:
        print(f"trace: start failed: {e!r}")
        return None


def _stop_trace(tracer, nc):
    """Stop the capture, convert NTFF -> perfetto, stash BassKernelResults."""
    lib, outdir = tracer
    try:
        n = lib.axon_stop_nrt_profile(str(outdir).encode())
        print(f"trace: {n} file(s) written to {outdir}")
        if n <= 0:
            return
        import gauge.profiler
        from concourse import bass_utils
        from concourse._compat import FishPath

        profile = gauge.profiler.Profile(
            profile_path=FishPath(outdir),
            kernel_dev_mode=True,
            profile_on_exit=False,
            bass_kernel=nc.m,
            offline_processing=True,
            fname="*_body*",
            metadata={},
        )
        perf = bass_utils._process_ntff_profile(
            profile,
            outdir,
            nc,
            core_ids=list(range(B)),
            trace_cores=[0],
            stitch_traces=False,
            trace_kwargs={},
            trace_events=False,
        )
        _nc_cache["last_results"] = perf.as_bass_kernel_results([])
    except Exception as e:
        print(f"trace: postprocess failed: {e!r}")


def _run_spmd_sharded(nc, in_maps, n_cores):
    """Like bass2jax.run_bass_via_pjrt, but moves data per-shard (16MB max per
    transfer) instead of one big concatenated host<->device transfer, which
    overflows the axon tunnel at our sizes (128MB outputs)."""
    import jax
    import jax.numpy as jnp
    import concourse.mybir as mybir
    from concourse import bass2jax
    from jax.sharding import Mesh, NamedSharding, PartitionSpec
    from jax.experimental.shard_map import shard_map

    bass2jax.install_neuronx_cc_hook()

    partition_name = nc.partition_id_tensor.name if nc.partition_id_tensor else None

    in_names, out_names, out_avals = [], [], []
    for alloc in nc.m.functions[0].allocations:
        if not isinstance(alloc, mybir.MemoryLocationSet):
            continue
        name = alloc.memorylocations[0].name
        if alloc.kind == "ExternalInput":
            if name != partition_name:
                in_names.append(name)
        elif alloc.kind == "ExternalOutput":
            out_names.append(name)
            out_avals.append(
                jax.core.ShapedArray(
                    tuple(alloc.tensor_shape), mybir.dt.np(alloc.dtype)
                )
            )
    n_params = len(in_names)
    n_outs = len(out_avals)
    in_names = in_names + out_names
    if partition_name is not None:
        in_names.append(partition_name)
    donate = tuple(range(n_params, n_params + n_outs))

    def _body(*args):
        operands = list(args)
        if partition_name is not None:
            operands.append(bass2jax.partition_id_tensor())
        return tuple(
            bass2jax._bass_exec_p.bind(
                *operands,
                out_avals=tuple(out_avals),
                in_names=tuple(in_names),
                out_names=tuple(out_names),
                lowering_input_output_aliases=(),
                sim_require_finite=True,
                sim_require_nnan=True,
                nc=nc,
            )
        )

    devices = jax.devices()[:n_cores]
    mesh = Mesh(np.asarray(devices), ("core",))
    sharding = NamedSharding(mesh, PartitionSpec("core"))
    in_specs = (PartitionSpec("core"),) * (n_params + n_outs)
    out_specs = (PartitionSpec("core"),) * n_outs
    fn = jax.jit(
        shard_map(
            _body, mesh=mesh, in_specs=in_specs, out_specs=out_specs, check_rep=False
        ),
        donate_argnums=donate,
        keep_unused=True,
    )

    def put(name):
        shards = [
            jax.device_put(np.asarray(in_maps[c][name]), devices[c])
            for c in range(n_cores)
        ]
        shp = shards[0].shape
        return jax.make_array_from_single_device_arrays(
            (n_cores * shp[0], *shp[1:]), sharding, shards
        )

    args = [put(name) for name in in_names[:n_params]]
    zeros = [
        jnp.zeros((n_cores * a.shape[0], *a.shape[1:]), a.dtype, device=sharding)
        for a in out_avals
    ]

    tracer = _start_trace() if os.environ.get("MINGRU_TRACE") == "1" else None
    out_arrs = fn(*args, *zeros)
    jax.block_until_ready(out_arrs)
    if tracer is not None:
        _stop_trace(tracer, nc)

    results = [dict() for _ in range(n_cores)]
    for i, name in enumerate(out_names):
        shards = sorted(
            out_arrs[i].addressable_shards, key=lambda s: s.index[0].start or 0
        )
        assert len(shards) == n_cores
        for c in range(n_cores):
            results[c][name] = np.asarray(shards[c].data)
    return results


def kernel(x, Wg, bg, Wv, bv, Wd, bd):
    x = np.asarray(x)
    nc = _build_nc()

    # W.T in [d, e] layout, stacked j: 0=decay, 1=value, 2=gate
    wt = np.stack(
        [
            np.ascontiguousarray(np.asarray(Wd).T),
            np.ascontiguousarray(np.asarray(Wv).T),
            np.ascontiguousarray(np.asarray(Wg).T),
        ]
    ).astype(_BF16)
    bv = np.asarray(bv)
    bias = np.stack(
        [np.asarray(bd), bv, np.asarray(bg), -bv]
    ).astype(np.float32)

    in_maps = []
    for b in range(B):
        xt = np.ascontiguousarray(x[b].T).astype(_BF16)  # [D, S]
        in_maps.append({"xt": xt, "wt": wt, "bias": bias})

    results = _run_spmd_sharded(nc, in_maps, n_cores=B)

    out = np.empty((B, S, D), np.float32)
    for b in range(B):
        out[b] = results[b]["out"].T
    return out

